# revision 24
# baseline (speedup 1.0000x reference)
"""NT-Xent contrastive loss on 8 Trainium2 NeuronCores (V4).

Math (reference): z = [z_i; z_j] (N=8192, D=128), zn = z/||z||,
sim = zn@zn.T / 0.1.  Row loss_i = logsumexp_{j!=i} sim[i,j] - sim[i, pos(i)],
loss = mean_i loss_i.

Sharding: rolled-column trick.  Core c receives z rolled by -1024*c rows.
Its 1024 local rows are rolled rows 0..1023; the self column of local row
i is i and the positive column is i + 4096 on EVERY core, so a single
static SPMD program works with no collectives.  The self logit is
suppressed by adding -5 to the diagonal cosine (logit -40).  Host sums
the 8 partial means.

V4 (from V2 baseline 119-143us):
  - ACT (exp) is the bottleneck: 8.4M exps/core at 1 elem/cycle/lane
    (1.2 GHz) = 55us floor.  Everything else is arranged to keep ACT
    100% busy from ~8us onward.
  - bf16 zn/znT: PE transposes and LDWEIGHTS at 1 cyc/row, matmul same
    speed as f32r, half the SBUF traffic.  (DMA-XBAR transpose was tried
    and is 10x slower than the cost model on HW: 1.24us/tile.)
  - Batch-wide DVE ops (one tensor_mul / 3D reduce / broadcast-mul per
    1024-row half-batch) instead of per-128-row-tile ops.
  - Column chunks [1024,1024,1536x4]: chunk 0 only needs the first 1024
    rows normalized+transposed, so the exp pipeline starts early; 6KB
    matmul PSUM tiles double-buffered + dedicated 4KB transpose PSUM
    tile = exactly 16KB, so transposes never steal matmul buffers.
  - Early dummy Exp pins the ACT table load into the DMA wait.
"""

import os
import sys

import numpy as np

_TRN_REPO = "/opt/trn_rl_repo"
if _TRN_REPO not in sys.path:
    sys.path.insert(0, _TRN_REPO)

from concourse import bacc, bass, mybir, tile
from concourse.bass_utils import run_bass_kernel_spmd

B = 4096
D = 128
N = 2 * B
N_CORES = 8
RPC = N // N_CORES  # 1024 rows per core
INV_T = 10.0
DIAG_SHIFT = -5.0

NBATCH = 4   # stage-A batches of 2048 rows
TPB = 16     # 128-row tiles per batch
RB = 8       # row blocks per core (128 rows each)
CW = [1024, 1024, 1536, 1536, 1536, 1536]   # chunk widths
CO = [0, 1024, 2048, 3584, 5120, 6656]      # chunk col offsets
QB = len(CW)
POSQ = 3          # chunk containing cols 4096..5120 (positive pairs)
POSOFF = 4096 - CO[POSQ]

_cache: dict = {}

def build():
    f32 = mybir.dt.float32
    bf16 = mybir.dt.bfloat16
    AX = mybir.AxisListType
    AF = mybir.ActivationFunctionType

    nc = bacc.Bacc(
        "TRN2", target_bir_lowering=False, debug=False, num_devices=N_CORES
    )

    # Pin Ln/Exp/Copy to one ACT table (see V2 note): strip this kernel's
    # funcs from every other table so there is a single ACT_TABLE_LOAD.
    tabs = bacc.get_activation_tables(nc.m.arch)
    pinned = set(tabs["natural_log_exp_and_others"])
    for k in tabs:
        if k != "natural_log_exp_and_others":
            tabs[k] = tabs[k] - pinned

    z_dram = nc.dram_tensor("z_roll", [N, D], f32, kind="ExternalInput")
    loss_dram = nc.dram_tensor("loss_part", [1, 1], f32, kind="ExternalOutput")

    import ml_dtypes

    eye_np = np.eye(128, dtype=np.float32)
    eye_dram = nc.inline_tensor(eye_np, name="eye128")
    eyeb_dram = nc.inline_tensor(
        eye_np.astype(ml_dtypes.bfloat16), name="eyeb128"
    )
    negI_dram = nc.inline_tensor(
        (DIAG_SHIFT * eye_np).astype(np.float32), name="negI128"
    )
    ones_dram = nc.inline_tensor(
        np.full((128, 1), 1.0 / N, np.float32), name="ones128"
    )

    with tile.TileContext(nc) as tc:
        with (
            tc.tile_pool(name="const", bufs=1) as cpool,
            tc.tile_pool(name="zin", bufs=NBATCH) as zpool,
            tc.tile_pool(name="zn", bufs=2) as npool,
            tc.tile_pool(name="scr", bufs=2) as spool,
            tc.tile_pool(name="persist", bufs=1) as ppool,
            tc.tile_pool(name="psum", bufs=2, space=bass.MemorySpace.PSUM) as qpool,
            tc.tile_pool(name="tpsum", bufs=2, space=bass.MemorySpace.PSUM) as tpool,
        ):
            ones_sb = cpool.tile([128, 1], f32)
            eye_sb = cpool.tile([128, 128], f32)
            eyeb_sb = cpool.tile([128, 128], bf16)
            negI_sb = cpool.tile([128, 128], f32)
            dum = cpool.tile([128, 1], f32)

            ssq = ppool.tile([128, NBATCH * TPB], f32)
            lnssq = ppool.tile([128, NBATCH * TPB], f32)
            inv = ppool.tile([128, NBATCH * TPB], f32)
            znT = ppool.tile([128, N], bf16)
            sexp = ppool.tile([128, RB, QB], f32)
            pos = ppool.tile([128, RB], f32)

            # Input DMAs: 4 sub-DMAs of 512 rows per batch.  Batch 0's are
            # split across the gpsimd and sync queues so rows 0..1023 land
            # after ~1 sub-DMA time; later batches stream on gpsimd.
            zin_tiles = []
            for b in range(NBATCH):
                zin = zpool.tile([128, TPB, 128], f32, name=f"zin{b}", tag="zin")
                zin_tiles.append(zin)

            def sub_dma(eng, b, s):
                r0 = 2048 * b + 512 * s
                src = z_dram[r0 : r0 + 512, :].rearrange("(t p) d -> p t d", p=128)
                eng.dma_start(zin_tiles[b][:, 4 * s : 4 * s + 4, :], src)

            # Input DMAs: batch 0's odd sub-DMAs ride the scalar queue (a
            # HWDGE engine, idle this early) in parallel with gpsimd; the
            # sync queue is avoided (TileContext bookkeeping sems block it).
            # trigger the single ACT table load immediately: the dummy's
            # input comes from a memset (not a DMA'd const), so the load
            # finishes during the input-DMA wait instead of 5us into it.
            nc.vector.memset(dum[:], 1.0)
            nc.scalar.activation(dum[:], dum[:], AF.Exp)
            sub_dma(nc.gpsimd, 0, 0)
            sub_dma(nc.scalar, 0, 1)
            nc.gpsimd.dma_start(eyeb_sb[:], eyeb_dram[:])
            nc.gpsimd.dma_start(ones_sb[:], ones_dram[:])
            sub_dma(nc.gpsimd, 0, 2)
            sub_dma(nc.scalar, 0, 3)
            nc.gpsimd.dma_start(negI_sb[:], negI_dram[:])
            nc.gpsimd.dma_start(eye_sb[:], eye_dram[:])
            for b in range(1, NBATCH):
                for s in range(4):
                    sub_dma(nc.gpsimd, b, s)

            # ---- stage-A helpers; h = half-batch index 0..7 (1024 rows) ----
            scr_t: dict = {}
            zn_t: dict = {}

            def sqmul(h):
                b, o = divmod(h, 2)
                scr = spool.tile([128, 8, 128], f32, name=f"sq{h}", tag="sq")
                zin8 = zin_tiles[b][:, 8 * o : 8 * o + 8, :]
                nc.vector.tensor_mul(scr[:], zin8, zin8)
                scr_t[h] = scr

            def sqred(h):
                scr = scr_t.pop(h)
                j0 = 8 * h
                nc.vector.reduce_sum(ssq[:, j0 : j0 + 8], scr[:], axis=AX.X)

            def norms(arg):
                j0, w = arg
                nc.scalar.activation(
                    lnssq[:, j0 : j0 + w], ssq[:, j0 : j0 + w], AF.Ln
                )
                nc.scalar.activation(
                    inv[:, j0 : j0 + w], lnssq[:, j0 : j0 + w], AF.Exp, scale=-0.5
                )

            def scale(h):
                b, o = divmod(h, 2)
                zn = npool.tile([128, 8, 128], bf16, name=f"zn{h}", tag="zn")
                j0 = 8 * h
                inv_b = inv[:, j0 : j0 + 8].unsqueeze(2).broadcast_to([128, 8, 128])
                nc.vector.tensor_mul(
                    zn[:], zin_tiles[b][:, 8 * o : 8 * o + 8, :], inv_b
                )
                zn_t[h] = zn

            def tpose(h):
                zn = zn_t.pop(h)
                ps = tpool.tile([128, 1024], bf16, tag="tp")
                for t in range(8):
                    nc.tensor.transpose(
                        ps[:, 128 * t : 128 * (t + 1)], zn[:, t, :], eyeb_sb[:]
                    )
                c0 = 1024 * h
                nc.vector.tensor_copy(znT[:, c0 : c0 + 1024], ps[:])

            # slot table: stage-A ops emitted after row-block r of chunk q.
            # Halves h1..h7 built during the main loop (h0 in prologue);
            # norms batched per 2048-row batch (one Ln+Exp pair instead of
            # two).  Transpose deadlines: t_h must land before the first
            # chunk reading znT cols [1024h, 1024h+1024).
            SLOTS = {
                (0, 0): [("m", 1)], (0, 1): [("r", 1)],
                (0, 2): [("n", (8, 8))], (0, 3): [("s", 1)],
                (0, 4): [("t", 1), ("m", 2)], (0, 5): [("r", 2)],
                (0, 6): [("m", 3)], (0, 7): [("r", 3)],
                (1, 0): [("n", (16, 16))], (1, 1): [("s", 2)],
                (1, 2): [("t", 2)], (1, 3): [("s", 3)],
                (1, 4): [("t", 3), ("m", 4)], (1, 5): [("r", 4)],
                (1, 6): [("m", 5)], (1, 7): [("r", 5)],
                (2, 0): [("n", (32, 16))], (2, 1): [("s", 4)],
                (2, 2): [("t", 4)], (2, 3): [("s", 5)],
                (2, 4): [("t", 5), ("m", 6)], (2, 5): [("r", 6)],
                (2, 6): [("m", 7)], (2, 7): [("r", 7)],
                (3, 0): [("n", (48, 16))], (3, 1): [("s", 6)],
                (3, 2): [("t", 6)], (3, 3): [("s", 7)],
                (3, 4): [("t", 7)],
            }
            FN = {"m": sqmul, "r": sqred, "s": scale, "t": tpose, "n": norms}

            def run_slot(q, r):
                for kind, arg in SLOTS.get((q, r), []):
                    FN[kind](arg)

            # PE p-state warmup: ~16 throwaway transposes keep the PE
            # continuously busy from ~6us so the first real matmuls run at
            # ramped clock instead of 0.65GHz cold.
            warm = tpool.tile([128, 512], bf16, tag="tp")
            for t in range(16):
                nc.tensor.transpose(
                    warm[:, 128 * (t % 4) : 128 * (t % 4 + 1)],
                    eyeb_sb[:], eyeb_sb[:],
                )

            # ---- prologue: half 0, in 512-row quarters so the first
            # matmul (chunk 0, k=0, cols 0..512) starts as early as possible
            for u in range(2):
                t0 = 4 * u
                scr = spool.tile([128, 4, 128], f32, name=f"psq{u}", tag="sq")
                zin4 = zin_tiles[0][:, t0 : t0 + 4, :]
                nc.vector.tensor_mul(scr[:], zin4, zin4)
                nc.vector.reduce_sum(ssq[:, t0 : t0 + 4], scr[:], axis=AX.X)
                norms((t0, 4))
                zn = npool.tile([128, 4, 128], bf16, name=f"pzn{u}", tag="zn")
                inv_b = inv[:, t0 : t0 + 4].unsqueeze(2).broadcast_to(
                    [128, 4, 128]
                )
                nc.vector.tensor_mul(zn[:], zin4, inv_b)
                ps = tpool.tile([128, 512], bf16, tag="tp")
                for t in range(4):
                    nc.tensor.transpose(
                        ps[:, 128 * t : 128 * (t + 1)], zn[:, t, :], eyeb_sb[:]
                    )
                nc.vector.tensor_copy(znT[:, 512 * u : 512 * (u + 1)], ps[:])

            # ---- main loop ----
            for q in range(QB):
                for r in range(RB):
                    w = CW[q]
                    lhsT = znT[:, 128 * r : 128 * (r + 1)]
                    ps = qpool.tile([128, w], f32, tag="mm")
                    for k in range(w // 512):
                        c0 = CO[q] + 512 * k
                        nc.tensor.matmul(
                            ps[:, 512 * k : 512 * (k + 1)],
                            lhsT,
                            znT[:, c0 : c0 + 512],
                            start=True,
                            stop=True,
                        )
                    if q == 0:
                        sub = ps[:, 128 * r : 128 * (r + 1)]
                        nc.vector.tensor_add(sub, sub, negI_sb[:])
                    if q == POSQ:
                        o0 = POSOFF + 128 * r
                        scr = spool.tile([128, 128], f32, name=f"px{r}", tag="sq")
                        nc.vector.tensor_mul(
                            scr[:], ps[:, o0 : o0 + 128], eye_sb[:]
                        )
                        nc.vector.reduce_sum(pos[:, r : r + 1], scr[:], axis=AX.X)
                    nc.scalar.activation(
                        ps[:],
                        ps[:],
                        AF.Exp,
                        scale=INV_T,
                        accum_out=sexp[:, r, q : q + 1],
                    )
                    run_slot(q, r)

            # ---- epilogue: loss = sum_r (lse_r - 10*pos_r) / N ----
            s8 = ppool.tile([128, RB], f32)
            nc.vector.reduce_sum(s8[:], sexp[:], axis=AX.X)
            lse = ppool.tile([128, RB], f32)
            nc.scalar.activation(lse[:], s8[:], AF.Ln)
            acc = ppool.tile([128, RB], f32)
            tot = ppool.tile([128, 1], f32)
            nc.vector.scalar_tensor_tensor(
                acc[:], pos[:], -INV_T, lse[:],
                op0=mybir.AluOpType.mult, op1=mybir.AluOpType.add,
                accum_out=tot[:],
            )
            # ones_sb holds 1/N, so the 1x1 matmul already yields the mean
            psf = qpool.tile([128, 512], f32, tag="mm")
            nc.tensor.matmul(
                psf[0:1, 0:1], ones_sb[:], tot[:], start=True, stop=True
            )
            res = ppool.tile([1, 1], f32)
            nc.vector.tensor_copy(res[:], psf[0:1, 0:1])
            nc.gpsimd.dma_start(loss_dram[:], res[:])

    nc.compile()
    return nc


def get_nc():
    if "nc" not in _cache:
        _cache["nc"] = build()
    return _cache["nc"]


def make_in_maps(z_i: np.ndarray, z_j: np.ndarray):
    z = np.concatenate(
        [np.asarray(z_i, np.float32), np.asarray(z_j, np.float32)], axis=0
    )
    return [
        {"z_roll": np.ascontiguousarray(np.roll(z, -RPC * c, axis=0))}
        for c in range(N_CORES)
    ]


def kernel(**inputs) -> np.ndarray:
    in_maps = make_in_maps(inputs["z_i"], inputs["z_j"])
    nc = get_nc()
    res = run_bass_kernel_spmd(nc, in_maps, list(range(N_CORES)))
    kernel.last_results = res
    total = np.float32(0.0)
    for r in res.results:
        total = np.float32(total + np.float32(np.asarray(r["loss_part"]).reshape(())))
    return np.float32(total)


# revision 25
# speedup vs baseline: 1.0002x; 1.0002x over previous
"""NT-Xent contrastive loss on 8 Trainium2 NeuronCores (V5, ~102us).

Math (reference): z = [z_i; z_j] (N=8192, D=128), zn = z/||z||,
sim = zn@zn.T / 0.1.  Row loss_i = logsumexp_{j!=i} sim[i,j] - sim[i, pos(i)],
loss = mean_i loss_i.

Sharding: rolled-column trick.  Core c receives z rolled by -1024*c rows.
Its 1024 local rows are rolled rows 0..1023; the self column of local row
i is i and the positive column is i + 4096 on EVERY core, so a single
static SPMD program works with no collectives.  The self logit is
suppressed by adding -5 to the diagonal cosine (logit -40).  Host sums
the 8 partial means.

Design (from V2 baseline 119-143us -> ~102us):
  - ACT (exp) is the hard bottleneck: 8.4M exps/core at 1 elem/cycle/lane
    (1.2 GHz) = 54.6us floor + ~10us accumulator reads + per-instruction
    init; measured ACT busy ~90us at 83% occupancy.
  - bf16 zn/znT: PE transposes and LDWEIGHTS at 1 cyc/row, matmuls same
    speed as f32r, half the SBUF traffic; rel err 4e-6 (gate is 2e-2).
  - Column chunks [1024,1024,1536x4]: chunk 0 needs only the first 1024
    rows normalized+transposed so exp starts ~8us in; 6KB matmul PSUM
    tiles double-buffered + 2x2KB transpose PSUM = exactly 16KB.
  - Batch-wide DVE ops; norms batched per 2048-row batch; stage-A work
    interleaved into exp slots via the SLOTS table.
  - Early memset-fed dummy Exp pins the single ACT table load into the
    DMA wait; 16 throwaway transposes ramp the PE p-state before use.
  - Epilogue: one scalar_tensor_tensor with accum_out, 1/N baked into
    the ones vector, DVE copy out of PSUM.

Measured dead-ends (do not retry without new evidence):
  - DMA-XBAR transpose (dma_start_transpose): 1.24us per 128x128 tile on
    HW, 10x the cost model; serializes on the issuing queue.
  - Symmetric-half (exp only upper-triangle blocks, exchange col-sums):
    needs a cross-core exchange, but a 4KB 8-core AllGather costs ~57us
    (ring hops ~15us each, seen in the instruction trace).  GPSIMD has
    no exp; TENSOR_REDUCE has no DVE fast modes.
  - bf16 PSUM matmul outputs: bass asserts matmul out must be fp32.
  - Custom-DVE Schraudolph exp (bit-shift trick, numerically safe):
    DVE datapath has no shift ops ("return 0.0 on trn2 HW") and no int
    add, and no polynomial spans e^+-10.  Exp cannot leave ACT.
"""

import os
import sys

import numpy as np

_TRN_REPO = "/opt/trn_rl_repo"
if _TRN_REPO not in sys.path:
    sys.path.insert(0, _TRN_REPO)

from concourse import bacc, bass, mybir, tile
from concourse.bass_utils import run_bass_kernel_spmd

B = 4096
D = 128
N = 2 * B
N_CORES = 8
RPC = N // N_CORES  # 1024 rows per core
INV_T = 10.0
DIAG_SHIFT = -5.0

NBATCH = 4   # stage-A batches of 2048 rows
TPB = 16     # 128-row tiles per batch
RB = 8       # row blocks per core (128 rows each)
CW = [1024, 1024, 1536, 1536, 1536, 1536]   # chunk widths
CO = [0, 1024, 2048, 3584, 5120, 6656]      # chunk col offsets
QB = len(CW)
POSQ = 3          # chunk containing cols 4096..5120 (positive pairs)
POSOFF = 4096 - CO[POSQ]

_cache: dict = {}

def build():
    f32 = mybir.dt.float32
    bf16 = mybir.dt.bfloat16
    AX = mybir.AxisListType
    AF = mybir.ActivationFunctionType

    nc = bacc.Bacc(
        "TRN2", target_bir_lowering=False, debug=False, num_devices=N_CORES
    )

    # Pin Ln/Exp/Copy to one ACT table (see V2 note): strip this kernel's
    # funcs from every other table so there is a single ACT_TABLE_LOAD.
    tabs = bacc.get_activation_tables(nc.m.arch)
    pinned = set(tabs["natural_log_exp_and_others"])
    for k in tabs:
        if k != "natural_log_exp_and_others":
            tabs[k] = tabs[k] - pinned

    z_dram = nc.dram_tensor("z_roll", [N, D], f32, kind="ExternalInput")
    loss_dram = nc.dram_tensor("loss_part", [1, 1], f32, kind="ExternalOutput")

    import ml_dtypes

    eye_np = np.eye(128, dtype=np.float32)
    eye_dram = nc.inline_tensor(eye_np, name="eye128")
    eyeb_dram = nc.inline_tensor(
        eye_np.astype(ml_dtypes.bfloat16), name="eyeb128"
    )
    negI_dram = nc.inline_tensor(
        (DIAG_SHIFT * eye_np).astype(np.float32), name="negI128"
    )
    ones_dram = nc.inline_tensor(
        np.full((128, 1), 1.0 / N, np.float32), name="ones128"
    )

    with tile.TileContext(nc) as tc:
        with (
            tc.tile_pool(name="const", bufs=1) as cpool,
            tc.tile_pool(name="zin", bufs=NBATCH) as zpool,
            tc.tile_pool(name="zn", bufs=2) as npool,
            tc.tile_pool(name="scr", bufs=2) as spool,
            tc.tile_pool(name="persist", bufs=1) as ppool,
            tc.tile_pool(name="psum", bufs=2, space=bass.MemorySpace.PSUM) as qpool,
            tc.tile_pool(name="tpsum", bufs=2, space=bass.MemorySpace.PSUM) as tpool,
        ):
            ones_sb = cpool.tile([128, 1], f32)
            eye_sb = cpool.tile([128, 128], f32)
            eyeb_sb = cpool.tile([128, 128], bf16)
            negI_sb = cpool.tile([128, 128], f32)
            dum = cpool.tile([128, 1], f32)

            ssq = ppool.tile([128, NBATCH * TPB], f32)
            lnssq = ppool.tile([128, NBATCH * TPB], f32)
            inv = ppool.tile([128, NBATCH * TPB], f32)
            znT = ppool.tile([128, N], bf16)
            sexp = ppool.tile([128, RB, QB], f32)
            pos = ppool.tile([128, RB], f32)

            # Input DMAs: 4 sub-DMAs of 512 rows per batch.  Batch 0's are
            # split across the gpsimd and sync queues so rows 0..1023 land
            # after ~1 sub-DMA time; later batches stream on gpsimd.
            zin_tiles = []
            for b in range(NBATCH):
                zin = zpool.tile([128, TPB, 128], f32, name=f"zin{b}", tag="zin")
                zin_tiles.append(zin)

            def sub_dma(eng, b, s):
                r0 = 2048 * b + 512 * s
                src = z_dram[r0 : r0 + 512, :].rearrange("(t p) d -> p t d", p=128)
                eng.dma_start(zin_tiles[b][:, 4 * s : 4 * s + 4, :], src)

            # Input DMAs: batch 0's odd sub-DMAs ride the scalar queue (a
            # HWDGE engine, idle this early) in parallel with gpsimd; the
            # sync queue is avoided (TileContext bookkeeping sems block it).
            # trigger the single ACT table load immediately: the dummy's
            # input comes from a memset (not a DMA'd const), so the load
            # finishes during the input-DMA wait instead of 5us into it.
            nc.vector.memset(dum[:], 1.0)
            nc.scalar.activation(dum[:], dum[:], AF.Exp)
            sub_dma(nc.gpsimd, 0, 0)
            sub_dma(nc.scalar, 0, 1)
            nc.gpsimd.dma_start(eyeb_sb[:], eyeb_dram[:])
            nc.gpsimd.dma_start(ones_sb[:], ones_dram[:])
            sub_dma(nc.gpsimd, 0, 2)
            sub_dma(nc.scalar, 0, 3)
            nc.gpsimd.dma_start(negI_sb[:], negI_dram[:])
            nc.gpsimd.dma_start(eye_sb[:], eye_dram[:])
            for b in range(1, NBATCH):
                for s in range(4):
                    sub_dma(nc.gpsimd, b, s)

            # ---- stage-A helpers; h = half-batch index 0..7 (1024 rows) ----
            scr_t: dict = {}
            zn_t: dict = {}

            def sqmul(h):
                b, o = divmod(h, 2)
                scr = spool.tile([128, 8, 128], f32, name=f"sq{h}", tag="sq")
                zin8 = zin_tiles[b][:, 8 * o : 8 * o + 8, :]
                nc.vector.tensor_mul(scr[:], zin8, zin8)
                scr_t[h] = scr

            def sqred(h):
                scr = scr_t.pop(h)
                j0 = 8 * h
                nc.vector.reduce_sum(ssq[:, j0 : j0 + 8], scr[:], axis=AX.X)

            def norms(arg):
                j0, w = arg
                nc.scalar.activation(
                    lnssq[:, j0 : j0 + w], ssq[:, j0 : j0 + w], AF.Ln
                )
                nc.scalar.activation(
                    inv[:, j0 : j0 + w], lnssq[:, j0 : j0 + w], AF.Exp, scale=-0.5
                )

            def scale(h):
                b, o = divmod(h, 2)
                zn = npool.tile([128, 8, 128], bf16, name=f"zn{h}", tag="zn")
                j0 = 8 * h
                inv_b = inv[:, j0 : j0 + 8].unsqueeze(2).broadcast_to([128, 8, 128])
                nc.vector.tensor_mul(
                    zn[:], zin_tiles[b][:, 8 * o : 8 * o + 8, :], inv_b
                )
                zn_t[h] = zn

            def tpose(h):
                zn = zn_t.pop(h)
                ps = tpool.tile([128, 1024], bf16, tag="tp")
                for t in range(8):
                    nc.tensor.transpose(
                        ps[:, 128 * t : 128 * (t + 1)], zn[:, t, :], eyeb_sb[:]
                    )
                c0 = 1024 * h
                nc.vector.tensor_copy(znT[:, c0 : c0 + 1024], ps[:])

            # slot table: stage-A ops emitted after row-block r of chunk q.
            # Halves h1..h7 built during the main loop (h0 in prologue);
            # norms batched per 2048-row batch (one Ln+Exp pair instead of
            # two).  Transpose deadlines: t_h must land before the first
            # chunk reading znT cols [1024h, 1024h+1024).
            SLOTS = {
                (0, 0): [("m", 1)], (0, 1): [("r", 1)],
                (0, 2): [("n", (8, 8))], (0, 3): [("s", 1)],
                (0, 4): [("t", 1), ("m", 2)], (0, 5): [("r", 2)],
                (0, 6): [("m", 3)], (0, 7): [("r", 3)],
                (1, 0): [("n", (16, 16))], (1, 1): [("s", 2)],
                (1, 2): [("t", 2)], (1, 3): [("s", 3)],
                (1, 4): [("t", 3), ("m", 4)], (1, 5): [("r", 4)],
                (1, 6): [("m", 5)], (1, 7): [("r", 5)],
                (2, 0): [("n", (32, 16))], (2, 1): [("s", 4)],
                (2, 2): [("t", 4)], (2, 3): [("s", 5)],
                (2, 4): [("t", 5), ("m", 6)], (2, 5): [("r", 6)],
                (2, 6): [("m", 7)], (2, 7): [("r", 7)],
                (3, 0): [("n", (48, 16))], (3, 1): [("s", 6)],
                (3, 2): [("t", 6)], (3, 3): [("s", 7)],
                (3, 4): [("t", 7)],
            }
            FN = {"m": sqmul, "r": sqred, "s": scale, "t": tpose, "n": norms}

            def run_slot(q, r):
                for kind, arg in SLOTS.get((q, r), []):
                    FN[kind](arg)

            # PE p-state warmup: ~16 throwaway transposes keep the PE
            # continuously busy from ~6us so the first real matmuls run at
            # ramped clock instead of 0.65GHz cold.
            warm = tpool.tile([128, 512], bf16, tag="tp")
            for t in range(16):
                nc.tensor.transpose(
                    warm[:, 128 * (t % 4) : 128 * (t % 4 + 1)],
                    eyeb_sb[:], eyeb_sb[:],
                )

            # ---- prologue: half 0, in 512-row quarters so the first
            # matmul (chunk 0, k=0, cols 0..512) starts as early as possible
            for u in range(2):
                t0 = 4 * u
                scr = spool.tile([128, 4, 128], f32, name=f"psq{u}", tag="sq")
                zin4 = zin_tiles[0][:, t0 : t0 + 4, :]
                nc.vector.tensor_mul(scr[:], zin4, zin4)
                nc.vector.reduce_sum(ssq[:, t0 : t0 + 4], scr[:], axis=AX.X)
                norms((t0, 4))
                zn = npool.tile([128, 4, 128], bf16, name=f"pzn{u}", tag="zn")
                inv_b = inv[:, t0 : t0 + 4].unsqueeze(2).broadcast_to(
                    [128, 4, 128]
                )
                nc.vector.tensor_mul(zn[:], zin4, inv_b)
                ps = tpool.tile([128, 512], bf16, tag="tp")
                for t in range(4):
                    nc.tensor.transpose(
                        ps[:, 128 * t : 128 * (t + 1)], zn[:, t, :], eyeb_sb[:]
                    )
                nc.vector.tensor_copy(znT[:, 512 * u : 512 * (u + 1)], ps[:])

            # ---- main loop ----
            for q in range(QB):
                for r in range(RB):
                    w = CW[q]
                    lhsT = znT[:, 128 * r : 128 * (r + 1)]
                    ps = qpool.tile([128, w], f32, tag="mm")
                    for k in range(w // 512):
                        c0 = CO[q] + 512 * k
                        nc.tensor.matmul(
                            ps[:, 512 * k : 512 * (k + 1)],
                            lhsT,
                            znT[:, c0 : c0 + 512],
                            start=True,
                            stop=True,
                        )
                    if q == 0:
                        sub = ps[:, 128 * r : 128 * (r + 1)]
                        nc.vector.tensor_add(sub, sub, negI_sb[:])
                    if q == POSQ:
                        o0 = POSOFF + 128 * r
                        scr = spool.tile([128, 128], f32, name=f"px{r}", tag="sq")
                        nc.vector.tensor_mul(
                            scr[:], ps[:, o0 : o0 + 128], eye_sb[:]
                        )
                        nc.vector.reduce_sum(pos[:, r : r + 1], scr[:], axis=AX.X)
                    nc.scalar.activation(
                        ps[:],
                        ps[:],
                        AF.Exp,
                        scale=INV_T,
                        accum_out=sexp[:, r, q : q + 1],
                    )
                    run_slot(q, r)

            # ---- epilogue: loss = sum_r (lse_r - 10*pos_r) / N ----
            s8 = ppool.tile([128, RB], f32)
            nc.vector.reduce_sum(s8[:], sexp[:], axis=AX.X)
            lse = ppool.tile([128, RB], f32)
            nc.scalar.activation(lse[:], s8[:], AF.Ln)
            acc = ppool.tile([128, RB], f32)
            tot = ppool.tile([128, 1], f32)
            nc.vector.scalar_tensor_tensor(
                acc[:], pos[:], -INV_T, lse[:],
                op0=mybir.AluOpType.mult, op1=mybir.AluOpType.add,
                accum_out=tot[:],
            )
            # ones_sb holds 1/N, so the 1x1 matmul already yields the mean
            psf = qpool.tile([128, 512], f32, tag="mm")
            nc.tensor.matmul(
                psf[0:1, 0:1], ones_sb[:], tot[:], start=True, stop=True
            )
            res = ppool.tile([1, 1], f32)
            nc.vector.tensor_copy(res[:], psf[0:1, 0:1])
            nc.gpsimd.dma_start(loss_dram[:], res[:])

    nc.compile()
    return nc


def get_nc():
    if "nc" not in _cache:
        _cache["nc"] = build()
    return _cache["nc"]


def make_in_maps(z_i: np.ndarray, z_j: np.ndarray):
    z = np.concatenate(
        [np.asarray(z_i, np.float32), np.asarray(z_j, np.float32)], axis=0
    )
    return [
        {"z_roll": np.ascontiguousarray(np.roll(z, -RPC * c, axis=0))}
        for c in range(N_CORES)
    ]


def kernel(**inputs) -> np.ndarray:
    in_maps = make_in_maps(inputs["z_i"], inputs["z_j"])
    nc = get_nc()
    res = run_bass_kernel_spmd(nc, in_maps, list(range(N_CORES)))
    kernel.last_results = res
    total = np.float32(0.0)
    for r in res.results:
        total = np.float32(total + np.float32(np.asarray(r["loss_part"]).reshape(())))
    return np.float32(total)
